# revision 11
# baseline (speedup 1.0000x reference)
"""EdgeEmbedding kernel for 8 Trainium2 NeuronCores.

y[e] = silu(concat(h[src[e]], h[tgt[e]], m[e]) @ W) / 0.6

Algebraic split: W = [W1; W2; W3] (rows 0:64, 64:128, 128:144), so
y = silu(T1[src] + T2[tgt] + m @ W3) / 0.6 with Tpair = h @ [W1 | W2]
precomputed per-atom on device (23x FLOP reduction vs per-edge matmul).

Sharding: edges data-parallel across 8 cores (250000 each, padded to
251904 = 123 groups x 2048); h / W / Tpair replicated per core.
Per edge, the two 256B half-rows of Tpair are fetched by indirect
(SWDGE) DMA gathers; m @ W3 runs on TensorE from host-pretransposed
stationary tiles; adds on VectorE; SiLU on ScalarE.
"""

import numpy as np

import concourse.bass as bass
import concourse.mybir as mybir
from concourse import bacc
from concourse.tile import TileContext
from concourse.bass_utils import run_bass_kernel_spmd

N_CORES = 8
NUM_ATOMS = 100000
A_PAD = 100096            # 782 * 128
E_TOTAL = 2000000
E_CORE = 250000
NG = 123                  # groups of 2048 edges per core
E_DEV = NG * 2048         # 251904
SCALE = 1.0 / 0.6
F32 = mybir.dt.float32
I32 = mybir.dt.int32

_PROG = None


def _build_program():
    nc = bacc.Bacc("TRN2", target_bir_lowering=False, debug=False)
    hT = nc.dram_tensor("hT", [64, A_PAD], F32, kind="ExternalInput")
    Wcat = nc.dram_tensor("Wcat", [64, 128], F32, kind="ExternalInput")
    W3 = nc.dram_tensor("W3", [16, 64], F32, kind="ExternalInput")
    src_i = nc.dram_tensor("src_i", [NG, 128, 16], I32, kind="ExternalInput")
    tgt_i = nc.dram_tensor("tgt_i", [NG, 128, 16], I32, kind="ExternalInput")
    m_st = nc.dram_tensor("m_st", [NG, 16, 2048], F32, kind="ExternalInput")
    out = nc.dram_tensor("out", [NG, 128, 16, 64], F32, kind="ExternalOutput")

    with TileContext(nc) as tc:
        with tc.tile_pool(name="dram", bufs=1, space="DRAM") as dpool:
            Tpair = dpool.tile([A_PAD, 128], F32)

            # ---- phase 1: Tpair[a] = [h @ W1 | h @ W2](a) ----
            with tc.tile_pool(name="ph1", bufs=2) as p1, \
                 tc.tile_pool(name="ps1", bufs=4, space="PSUM") as ps1, \
                 tc.tile_pool(name="wp", bufs=1) as wp:
                wcat_sb = wp.tile([64, 128], F32)
                nc.sync.dma_start(wcat_sb[:, :], Wcat[:, :])
                CH = 8192
                for c0 in range(0, A_PAD, CH):
                    na = min(CH, A_PAD - c0)
                    hTc = p1.tile([64, CH], F32, tag="hTc")
                    nc.sync.dma_start(hTc[:, :na], hT[:, c0:c0 + na])
                    for s in range(na // 128):
                        ps = ps1.tile([128, 128], F32)
                        nc.tensor.matmul(
                            out=ps[:, :],
                            lhsT=hTc[:, 128 * s:128 * s + 128],
                            rhs=wcat_sb[:, :],
                            start=True, stop=True)
                        tsb = p1.tile([128, 128], F32, tag="tsb")
                        nc.scalar.copy(tsb[:, :], ps[:, :])
                        a0 = c0 + 128 * s
                        nc.sync.dma_start(Tpair[a0:a0 + 128, :], tsb[:, :])

            tc.strict_bb_all_engine_barrier()

            # ---- phase 2: per 2048-edge group ----
            with tc.tile_pool(name="ip", bufs=3) as ip, \
                 tc.tile_pool(name="mp", bufs=3) as mp, \
                 tc.tile_pool(name="gp", bufs=2) as gp, \
                 tc.tile_pool(name="vp", bufs=4) as vp, \
                 tc.tile_pool(name="op", bufs=2) as op, \
                 tc.tile_pool(name="ps2", bufs=4, space="PSUM") as ps2, \
                 tc.tile_pool(name="wp2", bufs=1) as wp2:
                w3_sb = wp2.tile([16, 64], F32)
                nc.sync.dma_start(w3_sb[:, :], W3[:, :])
                for t in range(NG):
                    it_s = ip.tile([128, 16], I32, tag="its")
                    it_t = ip.tile([128, 16], I32, tag="itt")
                    nc.sync.dma_start(it_s[:, :], src_i[t])
                    nc.sync.dma_start(it_t[:, :], tgt_i[t])
                    mst = mp.tile([16, 2048], F32, tag="mst")
                    nc.sync.dma_start(mst[:, :], m_st[t])
                    gs = gp.tile([128, 16, 64], F32, tag="gs")
                    gt2 = gp.tile([128, 16, 64], F32, tag="gt")
                    for j in range(16):
                        nc.gpsimd.indirect_dma_start(
                            out=gs[:, j, :], out_offset=None,
                            in_=Tpair[:, :],
                            in_offset=bass.IndirectOffsetOnAxis(
                                ap=it_s[:, j:j + 1], axis=0),
                            element_offset=0)
                        nc.gpsimd.indirect_dma_start(
                            out=gt2[:, j, :], out_offset=None,
                            in_=Tpair[:, :],
                            in_offset=bass.IndirectOffsetOnAxis(
                                ap=it_t[:, j:j + 1], axis=0),
                            element_offset=64)
                    ot = op.tile([128, 16, 64], F32, tag="ot")
                    for b in range(8):
                        ps = ps2.tile([128, 2, 64], F32)
                        for hh in range(2):
                            j = 2 * b + hh
                            nc.tensor.matmul(
                                out=ps[:, hh, :],
                                lhsT=mst[:, 128 * j:128 * j + 128],
                                rhs=w3_sb[:, :],
                                start=True, stop=True)
                        y = vp.tile([128, 2, 64], F32, tag="y")
                        nc.vector.tensor_tensor(
                            out=y[:, :, :], in0=gs[:, 2 * b:2 * b + 2, :],
                            in1=gt2[:, 2 * b:2 * b + 2, :],
                            op=mybir.AluOpType.add)
                        nc.vector.tensor_tensor(
                            out=y[:, :, :], in0=y[:, :, :], in1=ps[:, :, :],
                            op=mybir.AluOpType.add)
                        z = vp.tile([128, 2, 64], F32, tag="z")
                        nc.scalar.activation(
                            out=z[:, :, :], in_=y[:, :, :],
                            func=mybir.ActivationFunctionType.Silu)
                        nc.vector.tensor_scalar_mul(
                            ot[:, 2 * b:2 * b + 2, :], z[:, :, :], SCALE)
                    nc.sync.dma_start(out[t], ot[:, :, :])
    nc.finalize()
    return nc


def _prepare_inputs(h, m, edge_index, W):
    h = np.asarray(h, dtype=np.float32)
    m = np.asarray(m, dtype=np.float32)
    W = np.asarray(W, dtype=np.float32)
    ei = np.asarray(edge_index).astype(np.int32)

    hT = np.zeros((64, A_PAD), dtype=np.float32)
    hT[:, :NUM_ATOMS] = h.T
    Wcat = np.concatenate([W[0:64, :], W[64:128, :]], axis=1).copy()
    W3 = W[128:144, :].copy()

    in_maps = []
    for c in range(N_CORES):
        lo, hi = c * E_CORE, (c + 1) * E_CORE
        n = min(E_CORE, E_DEV)
        src = np.zeros(E_DEV, dtype=np.int32)
        tgt = np.zeros(E_DEV, dtype=np.int32)
        src[:n] = ei[0, lo:lo + n]
        tgt[:n] = ei[1, lo:lo + n]
        mm = np.zeros((E_DEV, 16), dtype=np.float32)
        mm[:n] = m[lo:lo + n]
        # edge local index e = 2048 t + 128 j + p
        src_i = np.ascontiguousarray(
            src.reshape(NG, 16, 128).transpose(0, 2, 1))
        tgt_i = np.ascontiguousarray(
            tgt.reshape(NG, 16, 128).transpose(0, 2, 1))
        # m_st[t, f, 128 j + p] = m[2048 t + 128 j + p, f]
        mst = np.ascontiguousarray(
            mm.reshape(NG, 16, 128, 16).transpose(0, 3, 1, 2)
              .reshape(NG, 16, 2048))
        in_maps.append({"hT": hT, "Wcat": Wcat, "W3": W3,
                        "src_i": src_i, "tgt_i": tgt_i, "m_st": mst})
    return in_maps


def _run(inputs, trace=False):
    global _PROG
    if _PROG is None:
        _PROG = _build_program()
    in_maps = _prepare_inputs(**inputs)
    res = run_bass_kernel_spmd(
        _PROG, in_maps, core_ids=list(range(N_CORES)), trace=trace)
    outs = []
    for c in range(N_CORES):
        o = res.results[c]["out"]  # [NG, 128, 16, 64]
        o = o.transpose(0, 2, 1, 3).reshape(E_DEV, 64)[:E_CORE]
        outs.append(o)
    full = np.concatenate(outs, axis=0)
    return full, res


def kernel(h, m, edge_index, W):
    full, _ = _run(dict(h=h, m=m, edge_index=edge_index, W=W), trace=False)
    return full
